# revision 37
# baseline (speedup 1.0000x reference)
"""Trainium2 Bass kernel for nn_Block_31954556682442 (spiking MoE-SSA block).

Sharding: pure data-parallel over batch B=8 -> one sample (4 LIF time steps)
per NeuronCore, zero collectives. v4 design:
  - 1-term bf16 matmuls (BN scales folded into weight rows host-side)
  - biases folded into the LIF membrane update itself: t=0 via ScalarE
    Identity bias, t>0 via scalar_tensor_tensor (M + 2^t*b) + PSUM with a
    per-partition f32 bias table (no rank-1 bias matmuls except conv/fc2)
  - PSUM-direct LIF: membrane add reads PSUM in one DVE op, spike =
    ScalarE Sigmoid(2^30*(M-thr)) saturating to exact {0,1}
  - depthwise 3x3 conv = 9 accumulating matmuls with host-precomputed
    DIAGONAL stationary weights over zero-padded (18x18) spike frames
  - router-weighted expert sum = diagonal matmuls (diag built on DVE)
  - phase E software-pipelined: conv(ch-1) matmuls interleaved between
    fc1(ch) groups so the PE never waits on the same channel's LIF chain
Self-contained: hardcodes all shapes; no sibling imports.
"""
import numpy as np
import ml_dtypes

import concourse.bacc as bacc
import concourse.mybir as mybir
import concourse.tile as tile
from concourse.bass_utils import run_bass_kernel_spmd

F32 = mybir.dt.float32
BF16 = mybir.dt.bfloat16
AL = mybir.AluOpType
AF = mybir.ActivationFunctionType

T, B, C, N = 4, 8, 384, 256
ED = 96
NE = 4
NU = 5
HID, HH = 2048, 1024
S = float(1.0 / np.sqrt(1.0 + 1e-5))
P = 128
SIG = float(2.0 ** 30)   # sigmoid spike sharpness (saturates to exact {0,1})


def _body(nc, tc, d):
    from contextlib import ExitStack
    VE = nc.vector
    GE = nc.gpsimd
    SE = nc.scalar

    with ExitStack() as ctx:
        def pool(name, bufs, space="SBUF"):
            return ctx.enter_context(tc.tile_pool(name=name, bufs=bufs, space=space))

        wp = pool("wp", 1)
        psA = pool("psA", 6, "PSUM")
        psB = pool("psB", 2, "PSUM")
        xs_p = pool("xs_p", 3)       # (128,1024) f32 x, in-place residual
        xhi_p = pool("xhi_p", 3)     # (128,1024) bf16 2^t-scaled x
        xnhi_p = pool("xnhi_p", 3)   # (128,1024) bf16 2^t-scaled x_new
        m_p = pool("m_p", 1)         # persistent membranes
        sp_pl = pool("sp_pl", 4)     # kq spikes (96,1280) bf16
        vsp_p = pool("vsp_p", 4)     # v spikes (128,768) bf16
        wsp_p = pool("wsp_p", 4)     # router spikes (128,8) f32
        at_p = pool("at_p", 4)       # attn bf16 (128,256)
        rs_p = pool("rs_p", 4)       # res spikes (128,768) bf16
        dgy_p = pool("dgy_p", 32)    # y diag (128,128) bf16
        ys_p = pool("ys_p", 2)       # y bf16 (128,384)
        ydn_p = pool("ydn_p", 3)     # (128,1024) bf16
        fr_p = pool("fr_p", 3)       # padded frames (128,1296) bf16
        sp2_p = pool("sp2_p", 2)     # x2 spikes (128,1024) bf16
        mh_p = pool("mh_p", 8)       # (128,256) f32 h membranes
        mdw_p = pool("mdw_p", 2)     # (128,256) f32 dw membrane
        mg_p = pool("mg_p", 8)       # gated spikes (128,1024) bf16
        out_p = pool("out_p", 4)     # output staging (128,256) f32

        # ---------------- weight loads ----------------
        def wload(name, shape, dt=F32):
            w = wp.tile(shape, dt, name=name, tag=name)
            nc.sync.dma_start(out=w, in_=d[name])
            return w

        ident = wload('ident', [P, P], BF16)
        sgb = wload('sgb', [P, 4])   # column t = -SIG*2^t (sigmoid spike bias)

        def warm(n=1):
            pass

        xs = []
        for kt in range(3):
            x_ = xs_p.tile([P, 4 * N], F32, name=f"xs{kt}", tag="t")
            xs.append(x_)
        for kt in range(3):
            nc.sync.dma_start(out=xs[kt], in_=d['xin'][kt*P:(kt+1)*P, :])
        kqw = [wload(f'kq_w1_{kt}', [P, 480], BF16) for kt in range(3)]
        kqbt = wload('kqbt', [ED, 20])
        pjbt = wload('pjbt', [P, 12])
        f1bt = wload('f1bt', [P, 64])
        on5 = wload('ones512', [1, 512], BF16)
        vw = [wload(f'v_w1_{kt}', [P, 384], BF16) for kt in range(3)]
        rw = [wload(f'r_wT_{kt}', [P, 4], F32) for kt in range(3)]
        rb = wload('r_b', [1, 4])
        ones = wload('ones', [1, P])
        pjw = [wload(f'pj_w1_{kt}', [P, 384], BF16) for kt in range(3)]
        f1w = [wload(f'f1_w1_{kt}', [P, 2048], BF16) for kt in range(3)]
        dg = wload('dg', [P, 9216], BF16)
        bdw = wload('bdw', [1, 8 * P], BF16)
        f2w = [wload(f'f2_w1_{ch}', [P, 384], BF16) for ch in range(8)]
        f2b = wload('f2b', [1, 384], BF16)

        # membranes
        m_kq = m_p.tile([ED, NU * N], F32, name="m_kq", tag="m_kq")
        m_v = m_p.tile([P, 768], F32, name="m_v", tag="m_v")
        m_r = m_p.tile([P, 8], F32, name="m_r", tag="m_r")
        m_resA = m_p.tile([P, NE * 768], F32, name="m_resA", tag="m_resA")
        m_resB = m_p.tile([P, NE * 768], F32, name="m_resB", tag="m_resB")
        m_pj = m_p.tile([P, 768], F32, name="m_pj", tag="m_pj")
        m_o = m_p.tile([P, 768], F32, name="m_o", tag="m_o")

        # padded spike frames for dwconv (zeroed once; pads stay zero)
        frames = [fr_p.tile([P, 1296], BF16, name=f"frame{i}", tag="t")
                  for i in range(3)]
        for fr in frames:
            GE.memset(fr, 0)

        xhi = [xhi_p.tile([P, 4 * N], BF16, name=f"xhi{kt}", tag="t")
               for kt in range(3)]
        # PE warmup: flips HAM to K=8/8 before the first real matmuls
        pwarm = psB.tile([P, P], F32, name="pwarm", tag="pB")
        for wi in range(40):
            nc.tensor.matmul(pwarm, ident, ident, start=True, stop=True)
        warm_sink = wp.tile([P, 1], F32, name="warm_sink", tag="warm_sink")
        SE.activation(warm_sink, pwarm[:, 0:1], AF.Copy)

        # ---------------- phase A+B: kq / v / router matmuls + LIF ----------------
        sp_t = [sp_pl.tile([ED, NU * N], BF16, name=f"sp{t}", tag="t")
                for t in range(T)]
        vsp = [vsp_p.tile([P, 768], BF16, name=f"vsp{t}", tag="t")
               for t in range(T)]
        wsp = [wsp_p.tile([P, 8], F32, name=f"wsp{t}", tag="t")
               for t in range(T)]

        def kq_mms(tp):
            groups = []
            for u in range(NU):
                pt = psA.tile([ED, 512], F32, name=f"pkq{u}_{tp}", tag="pA")
                for kt in range(3):
                    nc.tensor.matmul(pt, kqw[kt][:, 96*u:96*(u+1)],
                                     xhi[kt][:, tp*512:(tp+1)*512],
                                     start=(kt == 0), stop=(kt == 2))
                groups.append(pt)
            return groups

        def kq_lif(srcs, t):
            ti = t % 2
            thr = float(2.0 ** t)
            for u in range(NU):
                col = u * 4 + t
                if t == 0:
                    SE.activation(m_kq[:, u*N:(u+1)*N],
                                  srcs[u][:, ti*N:(ti+1)*N], AF.Identity,
                                  bias=kqbt[:, col:col+1], scale=1.0)
                else:
                    VE.scalar_tensor_tensor(
                        out=m_kq[:, u*N:(u+1)*N], in0=m_kq[:, u*N:(u+1)*N],
                        scalar=kqbt[:, col:col+1],
                        in1=srcs[u][:, ti*N:(ti+1)*N],
                        op0=AL.add, op1=AL.add)
            VE.tensor_scalar(sp_t[t], m_kq, thr, None, AL.is_ge)
            if t < T - 1:
                VE.scalar_tensor_tensor(out=m_kq, in0=m_kq, scalar=thr,
                                        in1=m_kq, op0=AL.is_lt, op1=AL.mult)

        def v_mms(t):
            groups = []
            for mt in range(2):
                pv = psB.tile([P, 512], F32, name=f"pv{t}_{mt}", tag="pB")
                for kt in range(3):
                    nc.tensor.matmul(pv[:, 0:384],
                                     xhi[kt][:, t*N + mt*P: t*N + (mt+1)*P],
                                     vw[kt], start=(kt == 0), stop=(kt == 2))
                groups.append(pv)
            return groups

        def v_lif(groups, t):
            thr = float(2.0 ** t)
            for mt in range(2):
                if t == 0:
                    SE.activation(m_v[:, mt*384:(mt+1)*384], groups[mt][:, 0:384],
                                  AF.Copy)
                else:
                    VE.scalar_tensor_tensor(
                        out=m_v[:, mt*384:(mt+1)*384], in0=groups[mt][:, 0:384],
                        scalar=1.0, in1=m_v[:, mt*384:(mt+1)*384],
                        op0=AL.mult, op1=AL.add)
            VE.tensor_scalar(vsp[t], m_v, thr, None, AL.is_ge)
            if t < T - 1:
                VE.scalar_tensor_tensor(out=m_v, in0=m_v, scalar=thr,
                                        in1=m_v, op0=AL.is_lt, op1=AL.mult)

        def r_block(t):
            thr = float(2.0 ** t)
            for mt in range(2):
                pr = psB.tile([P, 512], F32, name=f"pr{t}_{mt}", tag="pB")
                for kt in range(3):
                    nc.tensor.matmul(pr[:, 0:4],
                                     xs[kt][:, t*N + mt*P: t*N + (mt+1)*P],
                                     rw[kt], start=(kt == 0), stop=False)
                nc.tensor.matmul(pr[:, 0:4], ones, rb, start=False, stop=True)
                if t == 0:
                    SE.activation(m_r[:, mt*4:(mt+1)*4], pr[:, 0:4], AF.Copy,
                                  bias=0.0, scale=thr)
                else:
                    VE.scalar_tensor_tensor(
                        out=m_r[:, mt*4:(mt+1)*4], in0=pr[:, 0:4], scalar=thr,
                        in1=m_r[:, mt*4:(mt+1)*4], op0=AL.mult, op1=AL.add)
            VE.tensor_scalar(wsp[t], m_r, thr, None, AL.is_ge)
            if t < T - 1:
                VE.scalar_tensor_tensor(out=m_r, in0=m_r, scalar=thr,
                                        in1=m_r, op0=AL.is_lt, op1=AL.mult)

        # ---------------- phase C helper ----------------
        ydn = [ydn_p.tile([P, 4 * N], BF16, name=f"ydn{dt}", tag="t")
               for dt in range(3)]

        dgy_t = [None] * T

        def build_diags(t):
            dgy = []
            for mt in range(2):
                for e in range(NE):
                    dq = dgy_p.tile([P, P], BF16, name=f"dgy{t}{mt}{e}", tag="t")
                    VE.scalar_tensor_tensor(
                        out=dq, in0=ident, scalar=wsp[t][:, mt*4+e:mt*4+e+1],
                        in1=ident, op0=AL.mult, op1=AL.bypass)
                    dgy.append(dq)
            dgy_t[t] = dgy

        def c_block(t):
            thr = float(2.0 ** t)
            dgy = dgy_t[t]
            rs_t = []
            for e in range(NE):
                at_sb = []
                for mt in range(2):
                    pa = psA.tile([P, 512], F32, name=f"pa{e}{t}{mt}", tag="pA")
                    nc.tensor.matmul(pa[:, 0:N], sp_t[t][:, mt*P:(mt+1)*P],
                                     sp_t[t][:, (1+e)*N:(2+e)*N],
                                     start=True, stop=True,
                                     skip_group_check=True)
                    ats = at_p.tile([P, N], BF16, name=f"at{e}{t}{mt}", tag="t")
                    SE.activation(ats, pa[:, 0:N], AF.Copy)
                    at_sb.append(ats)
                cur = m_resA if t % 2 == 0 else m_resB
                nxt = m_resB if t % 2 == 0 else m_resA
                for mt in range(2):
                    pr_ = psA.tile([P, 512], F32, name=f"pres{e}{t}{mt}", tag="pA")
                    for mk in range(2):
                        nc.tensor.matmul(pr_[:, 0:384],
                                         at_sb[mk][:, mt*P:(mt+1)*P],
                                         vsp[t][:, mk*384:(mk+1)*384],
                                         start=(mk == 0), stop=(mk == 1),
                                         skip_group_check=True)
                    sl = slice(e*768 + mt*384, e*768 + (mt+1)*384)
                    if t == 0:
                        SE.activation(cur[:, sl], pr_[:, 0:384], AF.Copy,
                                      bias=0.0, scale=0.5 * thr)
                    else:
                        VE.scalar_tensor_tensor(
                            out=cur[:, sl], in0=pr_[:, 0:384],
                            scalar=0.5 * thr, in1=cur[:, sl],
                            op0=AL.mult, op1=AL.add)
            for e in range(NE):
                cur = m_resA if t % 2 == 0 else m_resB
                nxt = m_resB if t % 2 == 0 else m_resA
                esl = slice(e*768, (e+1)*768)
                rs = rs_p.tile([P, 768], BF16, name=f"rs{e}{t}", tag="t")
                SE.activation(rs, cur[:, esl], AF.Sigmoid, bias=sgb[:, t:t+1],
                              scale=SIG)
                if t < T - 1:
                    # reset into the other buffer: no WAR against the spike read
                    VE.scalar_tensor_tensor(out=nxt[:, esl], in0=cur[:, esl],
                                            scalar=thr, in1=cur[:, esl],
                                            op0=AL.is_lt, op1=AL.mult)
                rs_t.append(rs)
            pys = [psA.tile([P, 512], F32, name=f"py{t}{mt}", tag="pA")
                   for mt in range(2)]
            for e in range(NE):
                for mt in range(2):
                    nc.tensor.matmul(pys[mt][:, 0:384], dgy[mt*4+e],
                                     rs_t[e][:, mt*384:(mt+1)*384],
                                     start=(e == 0), stop=(e == 3),
                                     skip_group_check=True)
            for mt in range(2):
                ys = ys_p.tile([P, 384], BF16, name=f"ys{t}{mt}", tag="t")
                SE.activation(ys, pys[mt][:, 0:384], AF.Copy)
                for dt in range(3):
                    ptr = psA.tile([P, P], BF16, name=f"ptr{t}{mt}{dt}", tag="pA")
                    nc.tensor.transpose(ptr, ys[:, dt*P:(dt+1)*P], ident)
                    SE.activation(ydn[dt][:, t*N + mt*P: t*N + (mt+1)*P], ptr,
                                  AF.Copy, bias=0.0, scale=thr)

        # ---------------- phase D helper ----------------
        xnhi = [xnhi_p.tile([P, 4 * N], BF16, name=f"xnhi{kt}", tag="t")
                for kt in range(3)]

        def d_block(tp):
            pps = []
            for mo in range(3):
                pp = psA.tile([P, 512], F32, name=f"pp{tp}{mo}", tag="pA")
                for kt in range(3):
                    nc.tensor.matmul(pp, pjw[kt][:, mo*P:(mo+1)*P],
                                     ydn[kt][:, tp*512:(tp+1)*512],
                                     start=(kt == 0), stop=(kt == 2))
                pps.append(pp)
            for ti in range(2):
                t = tp * 2 + ti
                thr = float(2.0 ** t)
                for mo in range(3):
                    col = mo * 4 + t
                    if t == 0:
                        SE.activation(m_pj[:, mo*N:(mo+1)*N],
                                      pps[mo][:, ti*N:(ti+1)*N], AF.Identity,
                                      bias=pjbt[:, col:col+1], scale=1.0)
                    else:
                        VE.scalar_tensor_tensor(
                            out=m_pj[:, mo*N:(mo+1)*N], in0=m_pj[:, mo*N:(mo+1)*N],
                            scalar=pjbt[:, col:col+1],
                            in1=pps[mo][:, ti*N:(ti+1)*N],
                            op0=AL.add, op1=AL.add)
                for mo in range(3):
                    VE.scalar_tensor_tensor(
                        out=xs[mo][:, t*N:(t+1)*N], in0=m_pj[:, mo*N:(mo+1)*N],
                        scalar=thr, in1=xs[mo][:, t*N:(t+1)*N],
                        op0=AL.is_ge, op1=AL.add)
                if t < T - 1:
                    VE.scalar_tensor_tensor(out=m_pj, in0=m_pj, scalar=thr,
                                            in1=m_pj, op0=AL.is_lt, op1=AL.mult)
                for mo in range(3):
                    SE.activation(xnhi[mo][:, t*N:(t+1)*N], xs[mo][:, t*N:(t+1)*N],
                                  AF.Copy, bias=0.0, scale=thr)

        # ---------------- phase E: fc1 + h-LIF + dwconv + dw-LIF + gate ----------
        # software pipeline: conv(ch-1) matmuls issue between fc1(ch) groups
        mgs = []
        ph_of = [None] * 8
        fr_of = [None] * 8
        sp2_of = [None] * 8
        pcs_of = [None] * 8

        mh_of = [None] * 8

        def fc1_mms_tp(ch, tp):
            grp = []
            for half in range(2):
                mth = ch + 8 * half
                pf = psA.tile([P, 512], F32, name=f"ph{ch}{half}{tp}",
                              tag="pA")
                for kt in range(3):
                    nc.tensor.matmul(pf, f1w[kt][:, mth*P:(mth+1)*P],
                                     xnhi[kt][:, tp*512:(tp+1)*512],
                                     start=(kt == 0), stop=(kt == 2))
                grp.append(pf)
            if tp == 0:
                ph_of[ch] = [grp, None]
            else:
                ph_of[ch][1] = grp

        def h_chain_part(ch, tp):
            ph = ph_of[ch][tp]
            if tp == 0:
                mh_of[ch] = tuple(mh_p.tile([P, N], F32, name=f"mh{j}{ch}",
                                            tag="t") for j in range(4))
                sp2_of[ch] = sp2_p.tile([P, 1024], BF16, name=f"sp2{ch}",
                                        tag="t")
                fr_of[ch] = frames[ch % 3].rearrange("p (t y x) -> p t y x",
                                                     t=4, y=18)
            mhq = mh_of[ch]
            sp2 = sp2_of[ch]
            fr4 = fr_of[ch]
            for ti in range(2):
                t = tp * 2 + ti
                thr = float(2.0 ** t)
                c1, c2 = mhq[2*(t % 2)], mhq[2*(t % 2)+1]
                n1, n2 = mhq[2*((t+1) % 2)], mhq[2*((t+1) % 2)+1]
                for half, m_hx in ((0, c1), (1, c2)):
                    col = (ch + 8 * half) * 4 + t
                    src = ph[half][:, ti*N:(ti+1)*N]
                    if t == 0:
                        SE.activation(m_hx, src, AF.Identity,
                                      bias=f1bt[:, col:col+1], scale=1.0)
                    else:
                        VE.scalar_tensor_tensor(
                            out=m_hx, in0=m_hx, scalar=f1bt[:, col:col+1],
                            in1=src, op0=AL.add, op1=AL.add)
                m1v = c1.rearrange("p (a y x) -> p a y x", a=1, y=16)
                SE.activation(fr4[:, t:t+1, 1:17, 1:17], m1v, AF.Sigmoid,
                              bias=sgb[:, t:t+1], scale=SIG)
                SE.activation(sp2[:, t*N:(t+1)*N], c2, AF.Sigmoid,
                              bias=sgb[:, t:t+1], scale=SIG)
                if t < T - 1:
                    VE.scalar_tensor_tensor(out=n1, in0=c1, scalar=thr,
                                            in1=c1, op0=AL.is_lt, op1=AL.mult)
                    VE.scalar_tensor_tensor(out=n2, in0=c2, scalar=thr,
                                            in1=c2, op0=AL.is_lt, op1=AL.mult)

        def conv_mms(ch):
            fr4 = fr_of[ch]
            pcs = []
            for tp in range(2):
                pc = psA.tile([P, 512], F32, name=f"pc{ch}{tp}", tag="pA")
                for i, (dy, dx) in enumerate([(a, b) for a in range(3)
                                              for b in range(3)]):
                    nc.tensor.matmul(pc, dg[:, (ch*9+i)*P:(ch*9+i+1)*P],
                                     fr4[:, tp*2:(tp+1)*2, dy:dy+16, dx:dx+16],
                                     start=(i == 0), stop=False)
                nc.tensor.matmul(pc, bdw[:, ch*P:(ch+1)*P], on5,
                                 start=False, stop=True)
                pcs.append(pc)
            pcs_of[ch] = pcs

        def dw_chain(ch):
            pcs = pcs_of[ch]
            sp2 = sp2_of[ch]
            m_dw = mdw_p.tile([P, N], F32, name=f"mdw{ch}", tag="t")
            mg = mg_p.tile([P, 1024], BF16, name=f"mg{ch}", tag="t")
            for t in range(T):
                ti, tp = t % 2, t // 2
                thr = float(2.0 ** t)
                if t == 0:
                    SE.activation(m_dw, pcs[tp][:, ti*N:(ti+1)*N], AF.Copy,
                                  bias=0.0, scale=thr)
                else:
                    VE.scalar_tensor_tensor(
                        out=m_dw, in0=pcs[tp][:, ti*N:(ti+1)*N], scalar=thr,
                        in1=m_dw, op0=AL.mult, op1=AL.add)
                VE.scalar_tensor_tensor(
                    out=mg[:, t*N:(t+1)*N], in0=m_dw, scalar=thr,
                    in1=sp2[:, t*N:(t+1)*N], op0=AL.is_ge, op1=AL.mult)
                if t < T - 1:
                    VE.scalar_tensor_tensor(out=m_dw, in0=m_dw, scalar=thr,
                                            in1=m_dw, op0=AL.is_lt, op1=AL.mult)
            mgs.append(mg)


        # ---------------- interleaved A/B/C/D flow ----------------
        for t in range(T):
            r_block(t)
            build_diags(t)
        # xhi copies issued after the router chain: the router's tiny t0
        # copies lead the ScalarE queue so its DVE chain starts immediately
        for kt in range(3):
            for t in range(T):
                SE.activation(xhi[kt][:, t*N:(t+1)*N], xs[kt][:, t*N:(t+1)*N],
                              AF.Copy, bias=0.0, scale=float(2.0 ** t))
        ka = kq_mms(0)
        kb = kq_mms(1)
        xkq1 = wp.tile([ED, 5 * 512], F32, name="xkq1", tag="xkq1")
        for u in range(NU):
            SE.activation(xkq1[:, u*512:(u+1)*512], kb[u], AF.Copy)
        kb_src = [xkq1[:, u*512:(u+1)*512] for u in range(NU)]
        va0 = v_mms(0)
        kq_lif(ka, 0)
        v_lif(va0, 0)
        va1 = v_mms(1)
        kq_lif(ka, 1)
        v_lif(va1, 1)
        c_block(0)
        va2 = v_mms(2)
        kq_lif(kb_src, 2)
        v_lif(va2, 2)
        c_block(1)
        d_block(0)
        va3 = v_mms(3)
        kq_lif(kb_src, 3)
        v_lif(va3, 3)
        c_block(2)
        c_block(3)
        d_block(1)

        for ch in range(9):
            if ch < 8:
                fc1_mms_tp(ch, 0)
            if ch >= 1:
                conv_mms(ch - 1)
            if ch < 8:
                h_chain_part(ch, 0)
            if ch < 8:
                fc1_mms_tp(ch, 1)
            if ch >= 1:
                dw_chain(ch - 1)
            if ch < 8:
                h_chain_part(ch, 1)

        # ---------------- phase F: fc2 + o-LIF + residual + store ----------------
        for tp in range(2):
            pos = []
            for mo in range(3):
                po = psA.tile([P, 512], F32, name=f"po{tp}{mo}", tag="pA")
                for ch in range(8):
                    nc.tensor.matmul(po, f2w[ch][:, mo*P:(mo+1)*P],
                                     mgs[ch][:, tp*512:(tp+1)*512],
                                     start=(ch == 0), stop=False)
                nc.tensor.matmul(po, f2b[:, mo*P:(mo+1)*P], on5,
                                 start=False, stop=True)
                pos.append(po)
            for ti in range(2):
                t = tp * 2 + ti
                thr = float(2.0 ** t)
                for mo in range(3):
                    if t == 0:
                        SE.activation(m_o[:, mo*N:(mo+1)*N],
                                      pos[mo][:, ti*N:(ti+1)*N], AF.Copy,
                                      bias=0.0, scale=thr)
                    else:
                        VE.scalar_tensor_tensor(
                            out=m_o[:, mo*N:(mo+1)*N],
                            in0=pos[mo][:, ti*N:(ti+1)*N],
                            scalar=thr, in1=m_o[:, mo*N:(mo+1)*N],
                            op0=AL.mult, op1=AL.add)
                    ot = out_p.tile([P, N], F32, name=f"ot{t}{mo}", tag="t")
                    VE.scalar_tensor_tensor(
                        out=ot, in0=m_o[:, mo*N:(mo+1)*N], scalar=thr,
                        in1=xs[mo][:, t*N:(t+1)*N], op0=AL.is_ge, op1=AL.add)
                    nc.sync.dma_start(out=d['out'][t*C + mo*P: t*C + (mo+1)*P, :],
                                      in_=ot)
                    if t < T - 1:
                        VE.scalar_tensor_tensor(
                            out=m_o[:, mo*N:(mo+1)*N], in0=m_o[:, mo*N:(mo+1)*N],
                            scalar=thr, in1=m_o[:, mo*N:(mo+1)*N],
                            op0=AL.is_lt, op1=AL.mult)


def _build():
    nc = bacc.Bacc()
    with tile.TileContext(nc) as tc:
        with tc.tile_pool(name="dram", bufs=1, space="DRAM") as dram:
            def din(name, shape, dt=F32):
                return dram.tile(shape, dt, kind="ExternalInput", name=name,
                                 uniquify=False)
            d = {
                'xin': din('xin', [C, 4 * N]),
                'out': dram.tile([T * C, N], F32, kind="ExternalOutput",
                                 name='out', uniquify=False),
                'ident': din('ident', [P, P], BF16),
                'sgb': din('sgb', [P, 4]),
                'kqbt': din('kqbt', [ED, 20]),
                'pjbt': din('pjbt', [P, 12]),
                'f1bt': din('f1bt', [P, 64]),
                'ones512': din('ones512', [1, 512], BF16),
                'r_b': din('r_b', [1, 4]),
                'ones': din('ones', [1, P]),
                'dg': din('dg', [P, 9216], BF16),
                'bdw': din('bdw', [1, 8 * P], BF16),
                'f2b': din('f2b', [1, 384], BF16),
            }
            for kt in range(3):
                d[f'kq_w1_{kt}'] = din(f'kq_w1_{kt}', [P, 480], BF16)
                d[f'v_w1_{kt}'] = din(f'v_w1_{kt}', [P, 384], BF16)
                d[f'r_wT_{kt}'] = din(f'r_wT_{kt}', [P, 4])
                d[f'pj_w1_{kt}'] = din(f'pj_w1_{kt}', [P, 384], BF16)
                d[f'f1_w1_{kt}'] = din(f'f1_w1_{kt}', [P, 2048], BF16)
            for ch in range(8):
                d[f'f2_w1_{ch}'] = din(f'f2_w1_{ch}', [P, 384], BF16)
            _body(nc, tc, d)
    nc.finalize()
    return nc


_NC_CACHE = {}


def _get_nc():
    if 'nc' not in _NC_CACHE:
        _NC_CACHE['nc'] = _build()
    return _NC_CACHE['nc']


def _bf(x):
    return np.ascontiguousarray(x.astype(ml_dtypes.bfloat16))


def _tcols(b):
    # (rows, k) -> (rows, k*4) with col u*4+t = b[:,u] * 2^t
    rows, k = b.shape
    out = np.empty((rows, k * 4), np.float32)
    for u in range(k):
        for t in range(4):
            out[:, u * 4 + t] = b[:, u] * (2.0 ** t)
    return out


def _prep_common(inputs):
    inp = {k: np.asarray(v, np.float32) for k, v in inputs.items()}
    # kq: [k | experts], BN scale folded into columns (out channels)
    kq_cols = [inp['k_w'].T * 0.5]
    kqb = [np.zeros(ED, np.float32)]
    for e in range(NE):
        kq_cols.append(inp['exp_w'][e].T * (0.5 * S * inp['exp_g'][e])[None, :])
        kqb.append(0.5 * inp['exp_b'][e])
    kq_w1 = np.concatenate(kq_cols, axis=1)            # (384, 480)
    kqbt = _tcols(np.stack(kqb, axis=1))               # (96, 20)
    pjb = 0.5 * (inp['proj_b'] * inp['proj_g'] * S + inp['proj_be'])
    pjbt = _tcols(pjb.reshape(3, P).T)                 # (128, 12)
    f1b = 0.5 * (inp['fc1_b'] * inp['fc1_g'] * S + inp['fc1_be'])
    f1bt = _tcols(f1b.reshape(16, P).T)                # (128, 64)
    # dwconv diagonal weights (128, 72*128) + bias row
    tap = inp['dw_w'][:, 0].reshape(HH, 9)             # (1024, 9)
    tap = tap * (0.5 * S * inp['dw_g'])[:, None]
    dgm = np.zeros((P, 72 * P), np.float32)
    for ch in range(8):
        for i in range(9):
            col = (ch * 9 + i) * P
            dgm[np.arange(P), col + np.arange(P)] = tap[ch*P:(ch+1)*P, i]
    bdw = (0.5 * (inp['dw_b'] * inp['dw_g'] * S + inp['dw_be'])).reshape(1, 8 * P)
    sgb = np.zeros((P, 4), np.float32)
    for t in range(T):
        sgb[:, t] = -SIG * (2.0 ** t)
    com = {
        'ident': np.eye(P, dtype=ml_dtypes.bfloat16),
        'sgb': sgb,
        'kqbt': np.ascontiguousarray(kqbt),
        'pjbt': np.ascontiguousarray(pjbt),
        'f1bt': np.ascontiguousarray(f1bt),
        'ones512': _bf(np.ones((1, 512), np.float32)),
        'r_b': np.ascontiguousarray(
            (0.5 * (inp['router_b'] * inp['router_g'] * S
                    + inp['router_be'])).reshape(1, 4)),
        'ones': np.ones((1, P), np.float32),
        'dg': _bf(dgm),
        'bdw': _bf(bdw),
        'f2b': _bf((0.5 * (inp['fc2_b'] * inp['fc2_g'] * S
                           + inp['fc2_be']))[None, :]),
    }
    v_w1 = inp['v_w'].T * 0.5
    r_wT = inp['router_w'].T * (inp['router_g'] * S * 0.5)[None, :]
    pj_w1 = inp['proj_w'].T * (0.5 * S * inp['proj_g'])[None, :]
    f1_w1 = inp['fc1_w'].T * (0.5 * S * inp['fc1_g'])[None, :]
    f2_w1 = inp['fc2_w'].T * (0.5 * S * inp['fc2_g'])[None, :]
    for kt in range(3):
        sl = slice(kt*P, (kt+1)*P)
        com[f'kq_w1_{kt}'] = _bf(kq_w1[sl])
        com[f'v_w1_{kt}'] = _bf(v_w1[sl])
        com[f'r_wT_{kt}'] = np.ascontiguousarray(r_wT[sl])
        com[f'pj_w1_{kt}'] = _bf(pj_w1[sl])
        com[f'f1_w1_{kt}'] = _bf(f1_w1[sl])
    for ch in range(8):
        com[f'f2_w1_{ch}'] = _bf(f2_w1[ch*P:(ch+1)*P])
    return com


def run(inputs, trace=False, tmpdir=None):
    com = _prep_common(inputs)
    x = np.asarray(inputs['x'], np.float32).reshape(T, B, C, N)
    in_maps = []
    for b in range(B):
        m = dict(com)
        m['xin'] = np.ascontiguousarray(x[:, b].transpose(1, 0, 2).reshape(C, T * N))
        in_maps.append(m)
    res = run_bass_kernel_spmd(_get_nc(), in_maps, list(range(B)),
                               trace=trace, tmpdir=tmpdir)
    out = np.empty((T, B, C, N), np.float32)
    for b in range(B):
        out[:, b] = res.results[b]['out'].reshape(T, C, N)
    return out.reshape(T * B, C, 16, 16), res.exec_time_ns


def kernel(**inputs):
    out, _ = run(inputs)
    return out


# revision 39
# speedup vs baseline: 1.0067x; 1.0067x over previous
"""Trainium2 Bass kernel for nn_Block_31954556682442 (spiking MoE-SSA block).

Sharding: pure data-parallel over batch B=8 -> one sample (4 LIF time steps)
per NeuronCore, zero collectives. v4 design:
  - 1-term bf16 matmuls (BN scales folded into weight rows host-side)
  - biases folded into the LIF membrane update itself: t=0 via ScalarE
    Identity bias, t>0 via scalar_tensor_tensor (M + 2^t*b) + PSUM with a
    per-partition f32 bias table (no rank-1 bias matmuls except conv/fc2)
  - PSUM-direct LIF: membrane add reads PSUM in one DVE op, spike =
    ScalarE Sigmoid(2^30*(M-thr)) saturating to exact {0,1}
  - depthwise 3x3 conv = 9 accumulating matmuls with host-precomputed
    DIAGONAL stationary weights over zero-padded (18x18) spike frames
  - router-weighted expert sum = diagonal matmuls (diag built on DVE)
  - phase E software-pipelined: conv(ch-1) matmuls interleaved between
    fc1(ch) groups so the PE never waits on the same channel's LIF chain
Self-contained: hardcodes all shapes; no sibling imports.
"""
import numpy as np
import ml_dtypes

import concourse.bacc as bacc
import concourse.mybir as mybir
import concourse.tile as tile
from concourse.bass_utils import run_bass_kernel_spmd

F32 = mybir.dt.float32
BF16 = mybir.dt.bfloat16
AL = mybir.AluOpType
AF = mybir.ActivationFunctionType

T, B, C, N = 4, 8, 384, 256
ED = 96
NE = 4
NU = 5
HID, HH = 2048, 1024
S = float(1.0 / np.sqrt(1.0 + 1e-5))
P = 128
SIG = float(2.0 ** 30)   # sigmoid spike sharpness (saturates to exact {0,1})


def _body(nc, tc, d):
    from contextlib import ExitStack
    VE = nc.vector
    GE = nc.gpsimd
    SE = nc.scalar

    with ExitStack() as ctx:
        def pool(name, bufs, space="SBUF"):
            return ctx.enter_context(tc.tile_pool(name=name, bufs=bufs, space=space))

        wp = pool("wp", 1)
        psA = pool("psA", 6, "PSUM")
        psB = pool("psB", 2, "PSUM")
        xs_p = pool("xs_p", 3)       # (128,1024) f32 x, in-place residual
        xhi_p = pool("xhi_p", 3)     # (128,1024) bf16 2^t-scaled x
        xnhi_p = pool("xnhi_p", 3)   # (128,1024) bf16 2^t-scaled x_new
        m_p = pool("m_p", 1)         # persistent membranes
        sp_pl = pool("sp_pl", 4)     # kq spikes (96,1280) bf16
        vsp_p = pool("vsp_p", 4)     # v spikes (128,768) bf16
        wsp_p = pool("wsp_p", 4)     # router spikes (128,8) f32
        at_p = pool("at_p", 4)       # attn bf16 (128,256)
        rs_p = pool("rs_p", 4)       # res spikes (128,768) bf16
        dgy_p = pool("dgy_p", 32)    # y diag (128,128) bf16
        ys_p = pool("ys_p", 2)       # y bf16 (128,384)
        ydn_p = pool("ydn_p", 3)     # (128,1024) bf16
        fr_p = pool("fr_p", 3)       # padded frames (128,1296) bf16
        sp2_p = pool("sp2_p", 2)     # x2 spikes (128,1024) bf16
        mh_p = pool("mh_p", 8)       # (128,256) f32 h membranes
        mdw_p = pool("mdw_p", 2)     # (128,256) f32 dw membrane
        mg_p = pool("mg_p", 8)       # gated spikes (128,1024) bf16
        out_p = pool("out_p", 4)     # output staging (128,256) f32

        # ---------------- weight loads ----------------
        def wload(name, shape, dt=F32):
            w = wp.tile(shape, dt, name=name, tag=name)
            nc.sync.dma_start(out=w, in_=d[name])
            return w

        ident = wload('ident', [P, P], BF16)
        sgb = wload('sgb', [P, 4])   # column t = -SIG*2^t (sigmoid spike bias)

        def warm(n=1):
            pass

        xs = []
        for kt in range(3):
            x_ = xs_p.tile([P, 4 * N], F32, name=f"xs{kt}", tag="t")
            xs.append(x_)
        for kt in range(3):
            nc.sync.dma_start(out=xs[kt], in_=d['xin'][kt*P:(kt+1)*P, :])
        kqw = [wload(f'kq_w1_{kt}', [P, 480], BF16) for kt in range(3)]
        kqbt = wload('kqbt', [ED, 20])
        pjbt = wload('pjbt', [P, 12])
        f1bt = wload('f1bt', [P, 64])
        on5 = wload('ones512', [1, 512], BF16)
        vw = [wload(f'v_w1_{kt}', [P, 384], BF16) for kt in range(3)]
        rw = [wload(f'r_wT_{kt}', [P, 4], F32) for kt in range(3)]
        rb = wload('r_b', [1, 4])
        ones = wload('ones', [1, P])
        pjw = [wload(f'pj_w1_{kt}', [P, 384], BF16) for kt in range(3)]
        f1w = [wload(f'f1_w1_{kt}', [P, 2048], BF16) for kt in range(3)]
        dg = wload('dg', [P, 9216], BF16)
        bdw = wload('bdw', [1, 8 * P], BF16)
        f2w = [wload(f'f2_w1_{ch}', [P, 384], BF16) for ch in range(8)]
        f2b = wload('f2b', [1, 384], BF16)

        # membranes
        m_kq = m_p.tile([ED, NU * N], F32, name="m_kq", tag="m_kq")
        m_vA = m_p.tile([P, 768], F32, name="m_vA", tag="m_vA")
        m_vB = m_p.tile([P, 768], F32, name="m_vB", tag="m_vB")
        m_r = m_p.tile([P, 8], F32, name="m_r", tag="m_r")
        m_resA = m_p.tile([P, NE * 768], F32, name="m_resA", tag="m_resA")
        m_resB = m_p.tile([P, NE * 768], F32, name="m_resB", tag="m_resB")
        m_pj = m_p.tile([P, 768], F32, name="m_pj", tag="m_pj")
        m_o = m_p.tile([P, 768], F32, name="m_o", tag="m_o")

        # padded spike frames for dwconv (zeroed once; pads stay zero)
        frames = [fr_p.tile([P, 1296], BF16, name=f"frame{i}", tag="t")
                  for i in range(3)]
        for fr in frames:
            GE.memset(fr, 0)

        # x bf16 copies with 2^t scaling per t-block
        xhi = [xhi_p.tile([P, 4 * N], BF16, name=f"xhi{kt}", tag="t")
               for kt in range(3)]
        for kt in range(3):
            for t in range(T):
                SE.activation(xhi[kt][:, t*N:(t+1)*N], xs[kt][:, t*N:(t+1)*N],
                              AF.Copy, bias=0.0, scale=float(2.0 ** t))

        # PE warmup right before the first real matmuls: flips HAM to K=8/8
        # while the xhi copies execute, so phase A starts on a warm clock
        pwarm = psB.tile([P, P], F32, name="pwarm", tag="pB")
        for wi in range(40):
            nc.tensor.matmul(pwarm, ident, ident, start=True, stop=True)
        warm_sink = wp.tile([P, 1], F32, name="warm_sink", tag="warm_sink")
        SE.activation(warm_sink, pwarm[:, 0:1], AF.Copy)

        # ---------------- phase A+B: kq / v / router matmuls + LIF ----------------
        sp_t = [sp_pl.tile([ED, NU * N], BF16, name=f"sp{t}", tag="t")
                for t in range(T)]
        vsp = [vsp_p.tile([P, 768], BF16, name=f"vsp{t}", tag="t")
               for t in range(T)]
        wsp = [wsp_p.tile([P, 8], F32, name=f"wsp{t}", tag="t")
               for t in range(T)]

        def kq_mms(tp):
            groups = []
            for u in range(NU):
                pt = psA.tile([ED, 512], F32, name=f"pkq{u}_{tp}", tag="pA")
                for kt in range(3):
                    nc.tensor.matmul(pt, kqw[kt][:, 96*u:96*(u+1)],
                                     xhi[kt][:, tp*512:(tp+1)*512],
                                     start=(kt == 0), stop=(kt == 2))
                groups.append(pt)
            return groups

        def kq_lif(srcs, t):
            ti = t % 2
            thr = float(2.0 ** t)
            for u in range(NU):
                col = u * 4 + t
                if t == 0:
                    SE.activation(m_kq[:, u*N:(u+1)*N],
                                  srcs[u][:, ti*N:(ti+1)*N], AF.Identity,
                                  bias=kqbt[:, col:col+1], scale=1.0)
                else:
                    VE.scalar_tensor_tensor(
                        out=m_kq[:, u*N:(u+1)*N], in0=m_kq[:, u*N:(u+1)*N],
                        scalar=kqbt[:, col:col+1],
                        in1=srcs[u][:, ti*N:(ti+1)*N],
                        op0=AL.add, op1=AL.add)
            VE.tensor_scalar(sp_t[t], m_kq, thr, None, AL.is_ge)
            if t < T - 1:
                VE.scalar_tensor_tensor(out=m_kq, in0=m_kq, scalar=thr,
                                        in1=m_kq, op0=AL.is_lt, op1=AL.mult)

        def v_mms(t):
            groups = []
            for mt in range(2):
                pv = psB.tile([P, 512], F32, name=f"pv{t}_{mt}", tag="pB")
                for kt in range(3):
                    nc.tensor.matmul(pv[:, 0:384],
                                     xhi[kt][:, t*N + mt*P: t*N + (mt+1)*P],
                                     vw[kt], start=(kt == 0), stop=(kt == 2))
                groups.append(pv)
            return groups

        def v_lif(groups, t):
            thr = float(2.0 ** t)
            cur = m_vA if t % 2 == 0 else m_vB
            nxt = m_vB if t % 2 == 0 else m_vA
            for mt in range(2):
                if t == 0:
                    SE.activation(cur[:, mt*384:(mt+1)*384], groups[mt][:, 0:384],
                                  AF.Copy)
                else:
                    VE.scalar_tensor_tensor(
                        out=cur[:, mt*384:(mt+1)*384], in0=groups[mt][:, 0:384],
                        scalar=1.0, in1=cur[:, mt*384:(mt+1)*384],
                        op0=AL.mult, op1=AL.add)
            SE.activation(vsp[t], cur, AF.Sigmoid, bias=sgb[:, t:t+1],
                          scale=SIG)
            if t < T - 1:
                # reset into the other buffer: no WAR against the spike read
                VE.scalar_tensor_tensor(out=nxt, in0=cur, scalar=thr,
                                        in1=cur, op0=AL.is_lt, op1=AL.mult)

        def r_block(t):
            thr = float(2.0 ** t)
            for mt in range(2):
                pr = psB.tile([P, 512], F32, name=f"pr{t}_{mt}", tag="pB")
                for kt in range(3):
                    nc.tensor.matmul(pr[:, 0:4],
                                     xs[kt][:, t*N + mt*P: t*N + (mt+1)*P],
                                     rw[kt], start=(kt == 0), stop=False)
                nc.tensor.matmul(pr[:, 0:4], ones, rb, start=False, stop=True)
                if t == 0:
                    SE.activation(m_r[:, mt*4:(mt+1)*4], pr[:, 0:4], AF.Copy,
                                  bias=0.0, scale=thr)
                else:
                    VE.scalar_tensor_tensor(
                        out=m_r[:, mt*4:(mt+1)*4], in0=pr[:, 0:4], scalar=thr,
                        in1=m_r[:, mt*4:(mt+1)*4], op0=AL.mult, op1=AL.add)
            VE.tensor_scalar(wsp[t], m_r, thr, None, AL.is_ge)
            if t < T - 1:
                VE.scalar_tensor_tensor(out=m_r, in0=m_r, scalar=thr,
                                        in1=m_r, op0=AL.is_lt, op1=AL.mult)

        # ---------------- phase C helper ----------------
        ydn = [ydn_p.tile([P, 4 * N], BF16, name=f"ydn{dt}", tag="t")
               for dt in range(3)]

        dgy_t = [None] * T

        def build_diags(t):
            dgy = []
            for mt in range(2):
                for e in range(NE):
                    dq = dgy_p.tile([P, P], BF16, name=f"dgy{t}{mt}{e}", tag="t")
                    VE.scalar_tensor_tensor(
                        out=dq, in0=ident, scalar=wsp[t][:, mt*4+e:mt*4+e+1],
                        in1=ident, op0=AL.mult, op1=AL.bypass)
                    dgy.append(dq)
            dgy_t[t] = dgy

        def c_block(t):
            thr = float(2.0 ** t)
            dgy = dgy_t[t]
            rs_t = []
            for e in range(NE):
                at_sb = []
                for mt in range(2):
                    pa = psA.tile([P, 512], F32, name=f"pa{e}{t}{mt}", tag="pA")
                    nc.tensor.matmul(pa[:, 0:N], sp_t[t][:, mt*P:(mt+1)*P],
                                     sp_t[t][:, (1+e)*N:(2+e)*N],
                                     start=True, stop=True,
                                     skip_group_check=True)
                    ats = at_p.tile([P, N], BF16, name=f"at{e}{t}{mt}", tag="t")
                    SE.activation(ats, pa[:, 0:N], AF.Copy)
                    at_sb.append(ats)
                cur = m_resA if t % 2 == 0 else m_resB
                nxt = m_resB if t % 2 == 0 else m_resA
                for mt in range(2):
                    pr_ = psA.tile([P, 512], F32, name=f"pres{e}{t}{mt}", tag="pA")
                    for mk in range(2):
                        nc.tensor.matmul(pr_[:, 0:384],
                                         at_sb[mk][:, mt*P:(mt+1)*P],
                                         vsp[t][:, mk*384:(mk+1)*384],
                                         start=(mk == 0), stop=(mk == 1),
                                         skip_group_check=True)
                    sl = slice(e*768 + mt*384, e*768 + (mt+1)*384)
                    if t == 0:
                        SE.activation(cur[:, sl], pr_[:, 0:384], AF.Copy,
                                      bias=0.0, scale=0.5 * thr)
                    else:
                        VE.scalar_tensor_tensor(
                            out=cur[:, sl], in0=pr_[:, 0:384],
                            scalar=0.5 * thr, in1=cur[:, sl],
                            op0=AL.mult, op1=AL.add)
            for e in range(NE):
                cur = m_resA if t % 2 == 0 else m_resB
                nxt = m_resB if t % 2 == 0 else m_resA
                esl = slice(e*768, (e+1)*768)
                rs = rs_p.tile([P, 768], BF16, name=f"rs{e}{t}", tag="t")
                SE.activation(rs, cur[:, esl], AF.Sigmoid, bias=sgb[:, t:t+1],
                              scale=SIG)
                if t < T - 1:
                    # reset into the other buffer: no WAR against the spike read
                    VE.scalar_tensor_tensor(out=nxt[:, esl], in0=cur[:, esl],
                                            scalar=thr, in1=cur[:, esl],
                                            op0=AL.is_lt, op1=AL.mult)
                rs_t.append(rs)
            pys = [psA.tile([P, 512], F32, name=f"py{t}{mt}", tag="pA")
                   for mt in range(2)]
            for e in range(NE):
                for mt in range(2):
                    nc.tensor.matmul(pys[mt][:, 0:384], dgy[mt*4+e],
                                     rs_t[e][:, mt*384:(mt+1)*384],
                                     start=(e == 0), stop=(e == 3),
                                     skip_group_check=True)
            for mt in range(2):
                ys = ys_p.tile([P, 384], BF16, name=f"ys{t}{mt}", tag="t")
                SE.activation(ys, pys[mt][:, 0:384], AF.Copy)
                for dt in range(3):
                    ptr = psA.tile([P, P], BF16, name=f"ptr{t}{mt}{dt}", tag="pA")
                    nc.tensor.transpose(ptr, ys[:, dt*P:(dt+1)*P], ident)
                    SE.activation(ydn[dt][:, t*N + mt*P: t*N + (mt+1)*P], ptr,
                                  AF.Copy, bias=0.0, scale=thr)

        # ---------------- phase D helper ----------------
        xnhi = [xnhi_p.tile([P, 4 * N], BF16, name=f"xnhi{kt}", tag="t")
                for kt in range(3)]

        def d_block(tp):
            pps = []
            for mo in range(3):
                pp = psA.tile([P, 512], F32, name=f"pp{tp}{mo}", tag="pA")
                for kt in range(3):
                    nc.tensor.matmul(pp, pjw[kt][:, mo*P:(mo+1)*P],
                                     ydn[kt][:, tp*512:(tp+1)*512],
                                     start=(kt == 0), stop=(kt == 2))
                pps.append(pp)
            for ti in range(2):
                t = tp * 2 + ti
                thr = float(2.0 ** t)
                for mo in range(3):
                    col = mo * 4 + t
                    if t == 0:
                        SE.activation(m_pj[:, mo*N:(mo+1)*N],
                                      pps[mo][:, ti*N:(ti+1)*N], AF.Identity,
                                      bias=pjbt[:, col:col+1], scale=1.0)
                    else:
                        VE.scalar_tensor_tensor(
                            out=m_pj[:, mo*N:(mo+1)*N], in0=m_pj[:, mo*N:(mo+1)*N],
                            scalar=pjbt[:, col:col+1],
                            in1=pps[mo][:, ti*N:(ti+1)*N],
                            op0=AL.add, op1=AL.add)
                for mo in range(3):
                    VE.scalar_tensor_tensor(
                        out=xs[mo][:, t*N:(t+1)*N], in0=m_pj[:, mo*N:(mo+1)*N],
                        scalar=thr, in1=xs[mo][:, t*N:(t+1)*N],
                        op0=AL.is_ge, op1=AL.add)
                if t < T - 1:
                    VE.scalar_tensor_tensor(out=m_pj, in0=m_pj, scalar=thr,
                                            in1=m_pj, op0=AL.is_lt, op1=AL.mult)
                for mo in range(3):
                    SE.activation(xnhi[mo][:, t*N:(t+1)*N], xs[mo][:, t*N:(t+1)*N],
                                  AF.Copy, bias=0.0, scale=thr)

        # ---------------- phase E: fc1 + h-LIF + dwconv + dw-LIF + gate ----------
        # software pipeline: conv(ch-1) matmuls issue between fc1(ch) groups
        mgs = []
        ph_of = [None] * 8
        fr_of = [None] * 8
        sp2_of = [None] * 8
        pcs_of = [None] * 8

        mh_of = [None] * 8

        def fc1_mms_tp(ch, tp):
            grp = []
            for half in range(2):
                mth = ch + 8 * half
                pf = psA.tile([P, 512], F32, name=f"ph{ch}{half}{tp}",
                              tag="pA")
                for kt in range(3):
                    nc.tensor.matmul(pf, f1w[kt][:, mth*P:(mth+1)*P],
                                     xnhi[kt][:, tp*512:(tp+1)*512],
                                     start=(kt == 0), stop=(kt == 2))
                grp.append(pf)
            if tp == 0:
                ph_of[ch] = [grp, None]
            else:
                ph_of[ch][1] = grp

        def h_chain_part(ch, tp):
            ph = ph_of[ch][tp]
            if tp == 0:
                mh_of[ch] = tuple(mh_p.tile([P, N], F32, name=f"mh{j}{ch}",
                                            tag="t") for j in range(4))
                sp2_of[ch] = sp2_p.tile([P, 1024], BF16, name=f"sp2{ch}",
                                        tag="t")
                fr_of[ch] = frames[ch % 3].rearrange("p (t y x) -> p t y x",
                                                     t=4, y=18)
            mhq = mh_of[ch]
            sp2 = sp2_of[ch]
            fr4 = fr_of[ch]
            for ti in range(2):
                t = tp * 2 + ti
                thr = float(2.0 ** t)
                c1, c2 = mhq[2*(t % 2)], mhq[2*(t % 2)+1]
                n1, n2 = mhq[2*((t+1) % 2)], mhq[2*((t+1) % 2)+1]
                for half, m_hx in ((0, c1), (1, c2)):
                    col = (ch + 8 * half) * 4 + t
                    src = ph[half][:, ti*N:(ti+1)*N]
                    if t == 0:
                        SE.activation(m_hx, src, AF.Identity,
                                      bias=f1bt[:, col:col+1], scale=1.0)
                    else:
                        VE.scalar_tensor_tensor(
                            out=m_hx, in0=m_hx, scalar=f1bt[:, col:col+1],
                            in1=src, op0=AL.add, op1=AL.add)
                m1v = c1.rearrange("p (a y x) -> p a y x", a=1, y=16)
                SE.activation(fr4[:, t:t+1, 1:17, 1:17], m1v, AF.Sigmoid,
                              bias=sgb[:, t:t+1], scale=SIG)
                SE.activation(sp2[:, t*N:(t+1)*N], c2, AF.Sigmoid,
                              bias=sgb[:, t:t+1], scale=SIG)
                if t < T - 1:
                    VE.scalar_tensor_tensor(out=n1, in0=c1, scalar=thr,
                                            in1=c1, op0=AL.is_lt, op1=AL.mult)
                    VE.scalar_tensor_tensor(out=n2, in0=c2, scalar=thr,
                                            in1=c2, op0=AL.is_lt, op1=AL.mult)

        def conv_mms(ch):
            fr4 = fr_of[ch]
            pcs = []
            for tp in range(2):
                pc = psA.tile([P, 512], F32, name=f"pc{ch}{tp}", tag="pA")
                for i, (dy, dx) in enumerate([(a, b) for a in range(3)
                                              for b in range(3)]):
                    nc.tensor.matmul(pc, dg[:, (ch*9+i)*P:(ch*9+i+1)*P],
                                     fr4[:, tp*2:(tp+1)*2, dy:dy+16, dx:dx+16],
                                     start=(i == 0), stop=False)
                nc.tensor.matmul(pc, bdw[:, ch*P:(ch+1)*P], on5,
                                 start=False, stop=True)
                pcs.append(pc)
            pcs_of[ch] = pcs

        def dw_chain(ch):
            pcs = pcs_of[ch]
            sp2 = sp2_of[ch]
            m_dw = mdw_p.tile([P, N], F32, name=f"mdw{ch}", tag="t")
            mg = mg_p.tile([P, 1024], BF16, name=f"mg{ch}", tag="t")
            for t in range(T):
                ti, tp = t % 2, t // 2
                thr = float(2.0 ** t)
                if t == 0:
                    SE.activation(m_dw, pcs[tp][:, ti*N:(ti+1)*N], AF.Copy,
                                  bias=0.0, scale=thr)
                else:
                    VE.scalar_tensor_tensor(
                        out=m_dw, in0=pcs[tp][:, ti*N:(ti+1)*N], scalar=thr,
                        in1=m_dw, op0=AL.mult, op1=AL.add)
                VE.scalar_tensor_tensor(
                    out=mg[:, t*N:(t+1)*N], in0=m_dw, scalar=thr,
                    in1=sp2[:, t*N:(t+1)*N], op0=AL.is_ge, op1=AL.mult)
                if t < T - 1:
                    VE.scalar_tensor_tensor(out=m_dw, in0=m_dw, scalar=thr,
                                            in1=m_dw, op0=AL.is_lt, op1=AL.mult)
            mgs.append(mg)


        # ---------------- interleaved A/B/C/D flow ----------------
        for t in range(T):
            r_block(t)
            build_diags(t)
        ka = kq_mms(0)
        kb = kq_mms(1)
        xkq1 = wp.tile([ED, 5 * 512], F32, name="xkq1", tag="xkq1")
        for u in range(NU):
            SE.activation(xkq1[:, u*512:(u+1)*512], kb[u], AF.Copy)
        kb_src = [xkq1[:, u*512:(u+1)*512] for u in range(NU)]
        va0 = v_mms(0)
        kq_lif(ka, 0)
        v_lif(va0, 0)
        va1 = v_mms(1)
        kq_lif(ka, 1)
        v_lif(va1, 1)
        c_block(0)
        va2 = v_mms(2)
        kq_lif(kb_src, 2)
        v_lif(va2, 2)
        c_block(1)
        d_block(0)
        va3 = v_mms(3)
        kq_lif(kb_src, 3)
        v_lif(va3, 3)
        c_block(2)
        c_block(3)
        d_block(1)

        for ch in range(9):
            if ch < 8:
                fc1_mms_tp(ch, 0)
            if ch >= 1:
                conv_mms(ch - 1)
            if ch < 8:
                h_chain_part(ch, 0)
            if ch < 8:
                fc1_mms_tp(ch, 1)
            if ch >= 1:
                dw_chain(ch - 1)
            if ch < 8:
                h_chain_part(ch, 1)

        # ---------------- phase F: fc2 + o-LIF + residual + store ----------------
        for tp in range(2):
            pos = []
            for mo in range(3):
                po = psA.tile([P, 512], F32, name=f"po{tp}{mo}", tag="pA")
                for ch in range(8):
                    nc.tensor.matmul(po, f2w[ch][:, mo*P:(mo+1)*P],
                                     mgs[ch][:, tp*512:(tp+1)*512],
                                     start=(ch == 0), stop=False)
                nc.tensor.matmul(po, f2b[:, mo*P:(mo+1)*P], on5,
                                 start=False, stop=True)
                pos.append(po)
            for ti in range(2):
                t = tp * 2 + ti
                thr = float(2.0 ** t)
                for mo in range(3):
                    if t == 0:
                        SE.activation(m_o[:, mo*N:(mo+1)*N],
                                      pos[mo][:, ti*N:(ti+1)*N], AF.Copy,
                                      bias=0.0, scale=thr)
                    else:
                        VE.scalar_tensor_tensor(
                            out=m_o[:, mo*N:(mo+1)*N],
                            in0=pos[mo][:, ti*N:(ti+1)*N],
                            scalar=thr, in1=m_o[:, mo*N:(mo+1)*N],
                            op0=AL.mult, op1=AL.add)
                    ot = out_p.tile([P, N], F32, name=f"ot{t}{mo}", tag="t")
                    VE.scalar_tensor_tensor(
                        out=ot, in0=m_o[:, mo*N:(mo+1)*N], scalar=thr,
                        in1=xs[mo][:, t*N:(t+1)*N], op0=AL.is_ge, op1=AL.add)
                    nc.sync.dma_start(out=d['out'][t*C + mo*P: t*C + (mo+1)*P, :],
                                      in_=ot)
                    if t < T - 1:
                        VE.scalar_tensor_tensor(
                            out=m_o[:, mo*N:(mo+1)*N], in0=m_o[:, mo*N:(mo+1)*N],
                            scalar=thr, in1=m_o[:, mo*N:(mo+1)*N],
                            op0=AL.is_lt, op1=AL.mult)


def _build():
    nc = bacc.Bacc()
    with tile.TileContext(nc) as tc:
        with tc.tile_pool(name="dram", bufs=1, space="DRAM") as dram:
            def din(name, shape, dt=F32):
                return dram.tile(shape, dt, kind="ExternalInput", name=name,
                                 uniquify=False)
            d = {
                'xin': din('xin', [C, 4 * N]),
                'out': dram.tile([T * C, N], F32, kind="ExternalOutput",
                                 name='out', uniquify=False),
                'ident': din('ident', [P, P], BF16),
                'sgb': din('sgb', [P, 4]),
                'kqbt': din('kqbt', [ED, 20]),
                'pjbt': din('pjbt', [P, 12]),
                'f1bt': din('f1bt', [P, 64]),
                'ones512': din('ones512', [1, 512], BF16),
                'r_b': din('r_b', [1, 4]),
                'ones': din('ones', [1, P]),
                'dg': din('dg', [P, 9216], BF16),
                'bdw': din('bdw', [1, 8 * P], BF16),
                'f2b': din('f2b', [1, 384], BF16),
            }
            for kt in range(3):
                d[f'kq_w1_{kt}'] = din(f'kq_w1_{kt}', [P, 480], BF16)
                d[f'v_w1_{kt}'] = din(f'v_w1_{kt}', [P, 384], BF16)
                d[f'r_wT_{kt}'] = din(f'r_wT_{kt}', [P, 4])
                d[f'pj_w1_{kt}'] = din(f'pj_w1_{kt}', [P, 384], BF16)
                d[f'f1_w1_{kt}'] = din(f'f1_w1_{kt}', [P, 2048], BF16)
            for ch in range(8):
                d[f'f2_w1_{ch}'] = din(f'f2_w1_{ch}', [P, 384], BF16)
            _body(nc, tc, d)
    nc.finalize()
    return nc


_NC_CACHE = {}


def _get_nc():
    if 'nc' not in _NC_CACHE:
        _NC_CACHE['nc'] = _build()
    return _NC_CACHE['nc']


def _bf(x):
    return np.ascontiguousarray(x.astype(ml_dtypes.bfloat16))


def _tcols(b):
    # (rows, k) -> (rows, k*4) with col u*4+t = b[:,u] * 2^t
    rows, k = b.shape
    out = np.empty((rows, k * 4), np.float32)
    for u in range(k):
        for t in range(4):
            out[:, u * 4 + t] = b[:, u] * (2.0 ** t)
    return out


def _prep_common(inputs):
    inp = {k: np.asarray(v, np.float32) for k, v in inputs.items()}
    # kq: [k | experts], BN scale folded into columns (out channels)
    kq_cols = [inp['k_w'].T * 0.5]
    kqb = [np.zeros(ED, np.float32)]
    for e in range(NE):
        kq_cols.append(inp['exp_w'][e].T * (0.5 * S * inp['exp_g'][e])[None, :])
        kqb.append(0.5 * inp['exp_b'][e])
    kq_w1 = np.concatenate(kq_cols, axis=1)            # (384, 480)
    kqbt = _tcols(np.stack(kqb, axis=1))               # (96, 20)
    pjb = 0.5 * (inp['proj_b'] * inp['proj_g'] * S + inp['proj_be'])
    pjbt = _tcols(pjb.reshape(3, P).T)                 # (128, 12)
    f1b = 0.5 * (inp['fc1_b'] * inp['fc1_g'] * S + inp['fc1_be'])
    f1bt = _tcols(f1b.reshape(16, P).T)                # (128, 64)
    # dwconv diagonal weights (128, 72*128) + bias row
    tap = inp['dw_w'][:, 0].reshape(HH, 9)             # (1024, 9)
    tap = tap * (0.5 * S * inp['dw_g'])[:, None]
    dgm = np.zeros((P, 72 * P), np.float32)
    for ch in range(8):
        for i in range(9):
            col = (ch * 9 + i) * P
            dgm[np.arange(P), col + np.arange(P)] = tap[ch*P:(ch+1)*P, i]
    bdw = (0.5 * (inp['dw_b'] * inp['dw_g'] * S + inp['dw_be'])).reshape(1, 8 * P)
    sgb = np.zeros((P, 4), np.float32)
    for t in range(T):
        sgb[:, t] = -SIG * (2.0 ** t)
    com = {
        'ident': np.eye(P, dtype=ml_dtypes.bfloat16),
        'sgb': sgb,
        'kqbt': np.ascontiguousarray(kqbt),
        'pjbt': np.ascontiguousarray(pjbt),
        'f1bt': np.ascontiguousarray(f1bt),
        'ones512': _bf(np.ones((1, 512), np.float32)),
        'r_b': np.ascontiguousarray(
            (0.5 * (inp['router_b'] * inp['router_g'] * S
                    + inp['router_be'])).reshape(1, 4)),
        'ones': np.ones((1, P), np.float32),
        'dg': _bf(dgm),
        'bdw': _bf(bdw),
        'f2b': _bf((0.5 * (inp['fc2_b'] * inp['fc2_g'] * S
                           + inp['fc2_be']))[None, :]),
    }
    v_w1 = inp['v_w'].T * 0.5
    r_wT = inp['router_w'].T * (inp['router_g'] * S * 0.5)[None, :]
    pj_w1 = inp['proj_w'].T * (0.5 * S * inp['proj_g'])[None, :]
    f1_w1 = inp['fc1_w'].T * (0.5 * S * inp['fc1_g'])[None, :]
    f2_w1 = inp['fc2_w'].T * (0.5 * S * inp['fc2_g'])[None, :]
    for kt in range(3):
        sl = slice(kt*P, (kt+1)*P)
        com[f'kq_w1_{kt}'] = _bf(kq_w1[sl])
        com[f'v_w1_{kt}'] = _bf(v_w1[sl])
        com[f'r_wT_{kt}'] = np.ascontiguousarray(r_wT[sl])
        com[f'pj_w1_{kt}'] = _bf(pj_w1[sl])
        com[f'f1_w1_{kt}'] = _bf(f1_w1[sl])
    for ch in range(8):
        com[f'f2_w1_{ch}'] = _bf(f2_w1[ch*P:(ch+1)*P])
    return com


def run(inputs, trace=False, tmpdir=None):
    com = _prep_common(inputs)
    x = np.asarray(inputs['x'], np.float32).reshape(T, B, C, N)
    in_maps = []
    for b in range(B):
        m = dict(com)
        m['xin'] = np.ascontiguousarray(x[:, b].transpose(1, 0, 2).reshape(C, T * N))
        in_maps.append(m)
    res = run_bass_kernel_spmd(_get_nc(), in_maps, list(range(B)),
                               trace=trace, tmpdir=tmpdir)
    out = np.empty((T, B, C, N), np.float32)
    for b in range(B):
        out[:, b] = res.results[b]['out'].reshape(T, C, N)
    return out.reshape(T * B, C, 16, 16), res.exec_time_ns


def kernel(**inputs):
    out, _ = run(inputs)
    return out
